# revision 2
# baseline (speedup 1.0000x reference)
"""Trainium2 Bass kernel v2 for nn_AttnConvolutionalDecoder.

Data-parallel over batch (2 per core, 8 cores). Major changes vs v1:
  - fp8 DoubleRow matmuls for the causal convs, res-proj and Gfold
    (halves PE streamed columns on the dominant contractions)
  - h carrier stored fp8 (scale SH); all scale factors fold into weights,
    biases, the sigmoid's ACT scale and identity-matmul constants
  - host-precomputed time-embedding parts (twB/twD tiles) injected into
    PSUM via identity matmuls, so every PSUM evacuation is a single-tensor
    op that the ACT engine can take
  - [128,1024] two-bank PSUM tiles (both time chunks per accumulation)
  - label parts via K=32 row-packed (tile_position) matmuls on a
    strip-replicated one-hot
  - attention normalizer folded: d' = dd * (SDP/den), ctx = d'@Gfold with
    enc2in+SH/SDP folded into Gfold
Engine split per layer: ACT: sigmoid, dd-evac, h-evac; DVE: t1a, cv2, recip;
Pool: d' only (no PSUM port on gpsimd).
"""

import numpy as np

L, KW, C, D, E = 4, 3, 512, 512, 512
T, B, S, V, MAXT = 1024, 16, 512, 32, 1024
NCORES = 8
BPC = B // NCORES
NCH, NC_T, P = 4, 2, 128
TC = T // NC_T
THP = T + 8          # h row: [0:2] zeros, [2:2+T] data, tail pad

SH = 64.0            # h storage scale (power of 2)
SW = 256.0           # conv weight scale
SDP = 256.0          # d' scale
CC = SW * SH         # conv psum carries CC * true value

USE_SW_INTERLEAVE = False

_compiled = None

# conv pair-block order: (tap, kc-pair-base) for glu, id, then res pairs
_PAIRS = [(0, 0), (0, 2), (1, 0), (1, 2), (2, 0), (2, 2)]


def _build_nc(reps=1):
    import concourse.bacc as bacc
    import concourse.mybir as mybir
    import concourse.tile as tile

    F32 = mybir.dt.float32
    BF16 = mybir.dt.bfloat16
    FP8 = mybir.dt.float8e4
    AF = mybir.ActivationFunctionType
    OP = mybir.AluOpType
    PM = (mybir.MatmulPerfMode.DoubleRowSwInterleave if USE_SW_INTERLEAVE
          else mybir.MatmulPerfMode.DoubleRow)

    nc = bacc.Bacc("TRN2", target_bir_lowering=False, debug=False,
                   num_devices=NCORES)
    dt = nc.dram_tensor

    Wconv8 = dt("Wconv8", [L, NCH, P, 22, 2, P], FP8, kind="ExternalInput").ap()
    W2e = dt("W2e", [L, NCH, P, NCH, P], BF16, kind="ExternalInput").ap()
    TwB = dt("TwB", [L, NCH, P, T], BF16, kind="ExternalInput").ap()
    TwD = dt("TwD", [L, NCH, P, T], BF16, kind="ExternalInput").ap()
    AL4 = dt("AL4", [L, P, P], BF16, kind="ExternalInput").ap()
    AIR4 = dt("AIR4", [L, P, P], BF16, kind="ExternalInput").ap()
    Ident = dt("Ident", [P, P], BF16, kind="ExternalInput").ap()
    IdentSW = dt("IdentSW", [P, P], BF16, kind="ExternalInput").ap()
    Wout = dt("Wout", [P, NCH, V], BF16, kind="ExternalInput").ap()
    Woutres = dt("Woutres", [P, NCH, V], BF16, kind="ExternalInput").ap()
    timeTb = dt("timeTb", [NCH, P, T], BF16, kind="ExternalInput").ap()
    lab4 = dt("lab4", [P, P], BF16, kind="ExternalInput").ap()
    oh4 = dt("oh4", [BPC, P, T], BF16, kind="ExternalInput").ap()
    enc_r = dt("enc_r", [BPC, NCH, P, E], BF16, kind="ExternalInput").ap()
    We2i_r = dt("We2i_r", [L, NCH, P, C], BF16, kind="ExternalInput").ap()
    onesv = dt("onesv", [P, 2], BF16, kind="ExternalInput").ap()
    # bias columns f32: per (i,m): [bid, bglu*CC, dummy]; plus out bias col
    NB = 2 * L * NCH + 1
    biasall = dt("biasall", [P, NB], F32, kind="ExternalInput").ap()

    out = dt("out", [BPC, V, T], F32, kind="ExternalOutput").ap()

    with tile.TileContext(nc) as tc, \
         nc.allow_low_precision(reason="fp8/bf16 pipeline validated offline"):
        from contextlib import ExitStack
        es = ExitStack()

        def pool(name, bufs, space="SBUF"):
            return es.enter_context(
                tc.tile_pool(name=name, bufs=bufs, space=space))

        pers = pool("pers", 1)
        dram = pool("dram", 1, space="DRAM")
        wp = pool("wp", 3)              # conv weight blocks [14,P,2,P] fp8
        w2p = pool("w2p", 8)           # misc weight tiles <=1KB/part
        twp = pool("twp", 14)           # twB/twD [P,T] bf16 tiles
        gfp = pool("gfp", 6)            # gfold tiles [P,2,C] fp8
        sgp = pool("sgp", 3)            # sg tiles [P,1024] bf16
        t1p = pool("t1p", 3)            # t1a tiles [P,1024] bf16
        ps2 = pool("ps2", 4, space="PSUM")   # [P,1024] f32 (2 banks each)

        def mm(o, lhsT, rhs, start, stop, pm=None, tp=None):
            nc.tensor.matmul(o, lhsT, rhs, start=start, stop=stop,
                             perf_mode=pm, tile_position=tp)

        # ---- persistent tiles ----
        h = [pers.tile([P, NCH, THP], BF16, tag=f"h_{b}", name=f"h_{b}")
             for b in range(BPC)]
        h8 = [pers.tile([P, NCH, THP], FP8, tag=f"h8_{b}", name=f"h8_{b}")
              for b in range(BPC)]
        h8l = [pers.tile([P, NCH, THP], FP8, tag=f"hl_{b}", name=f"hl_{b}")
               for b in range(BPC)]
        cv = [pers.tile([P, NCH, T], BF16, tag=f"cv_{b}", name=f"cv_{b}")
              for b in range(BPC)]
        ddt = [pers.tile([P, NCH, T], BF16, tag=f"dd_{b}", name=f"dd_{b}")
               for b in range(BPC)]
        dp = [pers.tile([P, NCH, T], FP8, tag=f"dp_{b}", name=f"dp_{b}")
              for b in range(BPC)]
        rbt = [pers.tile([P, T], BF16, tag=f"rb_{b}", name=f"rb_{b}")
               for b in range(BPC)]
        oh_t = [pers.tile([P, T], BF16, tag=f"oh_{b}", name=f"oh_{b}")
                for b in range(BPC)]
        mrep = [[pers.tile([P, P], BF16, tag=f"mr_{b}_{k}", name=f"mr_{b}_{k}")
                 for k in range(NCH)] for b in range(BPC)]
        id_t = pers.tile([P, P], BF16, tag="id", name="id")
        idsw_t = pers.tile([P, P], BF16, tag="idsw", name="idsw")
        ball = pers.tile([P, NB], F32, tag="ball", name="ball")
        nc.sync.dma_start(out=ball, in_=biasall)
        nc.sync.dma_start(out=id_t, in_=Ident)
        nc.sync.dma_start(out=idsw_t, in_=IdentSW)
        for b in range(BPC):
            nc.sync.dma_start(out=oh_t[b], in_=oh4[b])
            nc.vector.memset(h[b][:, :, 0:2], 0)
            nc.vector.memset(h8[b][:, :, 0:2], 0)
            nc.vector.memset(h8l[b][:, :, 0:2], 0)

        def bid_col(i, m):
            return ball[:, (i * NCH + m):(i * NCH + m) + 1]

        def bglu_col(i, m):
            idx = L * NCH + i * NCH + m
            return ball[:, idx:idx + 1]

        bout_col = ball[0:V, NB - 1:NB]

        # DRAM staging
        embd = [dram.tile([P, NCH, T], BF16, tag=f"embd_{b}", name=f"embd_{b}")
                for b in range(BPC)]
        embd8 = [dram.tile([P, NCH, T], FP8, tag=f"em8_{b}", name=f"em8_{b}")
                 for b in range(BPC)]
        embdS = [dram.tile([P, NCH, T], BF16, tag=f"emS_{b}", name=f"emS_{b}")
                 for b in range(BPC)]
        embd8l = [dram.tile([P, NCH, T], FP8, tag=f"el8_{b}", name=f"el8_{b}")
                  for b in range(BPC)]
        gfd = [[[dram.tile([P, 2, C], FP8, tag=f"gf_{i}_{b}_{p}",
                           name=f"gf_{i}_{b}_{p}") for p in range(2)]
                for b in range(BPC)] for i in range(L)]

        # ---- startup: emb, G, Gfold, mrep ----
        with tc.tile_pool(name="su", bufs=1) as su, \
             tc.tile_pool(name="sur", bufs=2) as sur:
            lw_t = su.tile([P, P], BF16, tag="lw", name="lw")
            nc.sync.dma_start(out=lw_t, in_=lab4)
            ones_t = su.tile([P, 2], BF16, tag="ones", name="ones")
            nc.sync.dma_start(out=ones_t, in_=onesv)
            tt = [su.tile([P, T], BF16, tag=f"tt{k}", name=f"tt{k}")
                  for k in range(NCH)]
            for k in range(NCH):
                nc.sync.dma_start(out=tt[k], in_=timeTb[k])
            # emb per (b, kd): onehot K=32 packed matmuls + timeT add
            for b in range(BPC):
                for kd in range(NCH):
                    pe = ps2.tile([P, T], F32, tag="ps", name="ps")
                    for ch in range(NC_T):
                        t0 = ch * TC
                        mm(pe[:, t0:t0 + TC],
                           lw_t[32 * kd:32 * (kd + 1), :],
                           oh_t[b][32 * kd:32 * (kd + 1), t0:t0 + TC],
                           True, False, tp=(32 * kd, 0))
                        mm(pe[:, t0:t0 + TC], id_t, tt[kd][:, t0:t0 + TC],
                           False, True)
                    eb = sur.tile([P, T], BF16, tag="eb", name="eb")
                    nc.vector.tensor_copy(out=eb, in_=pe)
                    nc.gpsimd.dma_start(out=embd[b][:, kd, :], in_=eb)
                    ebs = sur.tile([P, T], BF16, tag="ebs", name="ebs")
                    nc.scalar.mul(ebs, eb, SH)
                    nc.gpsimd.tensor_copy(out=h[b][:, kd, 2:2 + T], in_=ebs)
                    nc.gpsimd.dma_start(out=embdS[b][:, kd, :], in_=ebs)
                    e8 = sur.tile([P, T], FP8, tag="e8", name="e8")
                    nc.vector.tensor_copy(out=e8, in_=ebs)
                    nc.gpsimd.tensor_copy(out=h8[b][:, kd, 2:2 + T], in_=e8)
                    nc.gpsimd.dma_start(out=embd8[b][:, kd, :], in_=e8)
                    e8l = sur.tile([P, T], FP8, tag="e8l", name="e8l")
                    nc.vector.tensor_tensor(e8l, ebs, e8, OP.subtract)
                    nc.gpsimd.tensor_copy(out=h8l[b][:, kd, 2:2 + T], in_=e8l)
                    nc.gpsimd.dma_start(out=embd8l[b][:, kd, :], in_=e8l)
            # enc tiles; G[b][m] = enc_b^T enc_b chunks; mrep
            G = [[su.tile([P, E], BF16, tag=f"G{b}_{m}", name=f"G{b}_{m}")
                  for m in range(NCH)] for b in range(BPC)]
            for b in range(BPC):
                er = []
                for sc in range(NCH):
                    t = su.tile([P, E], BF16, tag=f"er{sc}", name=f"er{sc}")
                    nc.sync.dma_start(out=t, in_=enc_r[b, sc])
                    er.append(t)
                for m in range(NCH):
                    pg_ = ps2.tile([P, E], F32, tag="ps", name="ps")
                    for sc in range(NCH):
                        mm(pg_, er[sc][:, m * P:(m + 1) * P], er[sc],
                           sc == 0, sc == NCH - 1)
                    nc.vector.tensor_copy(out=G[b][m], in_=pg_)
                for kc in range(NCH):
                    pm_ = ps2.tile([P, 2], F32, tag="ps", name="ps")
                    for sc in range(NCH):
                        mm(pm_, er[sc][:, kc * P:(kc + 1) * P], ones_t,
                           sc == 0, sc == NCH - 1)
                    nc.scalar.copy(mrep[b][kc],
                                   pm_[:, 0:1].to_broadcast([P, P]))
            # Gfold -> fp8 DRAM, layout [pair, P(e-in-chunk), 2, C]
            for i in range(L):
                e2r = []
                for kc in range(NCH):
                    t = su.tile([P, C], BF16, tag=f"e2r{kc}", name=f"e2r{kc}")
                    nc.sync.dma_start(out=t, in_=We2i_r[i, kc])
                    e2r.append(t)
                for b in range(BPC):
                    for pair in range(2):
                        gtile = sur.tile([P, 2, C], FP8, tag="gt", name="gt")
                        for half in range(2):
                            j = 2 * pair + half      # e-chunk index
                            pf = ps2.tile([P, C], F32, tag="ps", name="ps")
                            for kc in range(NCH):
                                mm(pf, G[b][kc][:, j * P:(j + 1) * P],
                                   e2r[kc], kc == 0, kc == NCH - 1)
                            nc.vector.tensor_copy(out=gtile[:, half, :],
                                                  in_=pf)
                        nc.gpsimd.dma_start(out=gfd[i][b][pair], in_=gtile)

        # ---- main loop ----
        for rep in range(reps):
            if rep > 0:
                for b in range(BPC):
                    nc.sync.dma_start(out=h[b][:, :, 2:2 + T],
                                      in_=embdS[b])
                    nc.sync.dma_start(out=h8[b][:, :, 2:2 + T],
                                      in_=embd8[b])
                    nc.sync.dma_start(out=h8l[b][:, :, 2:2 + T],
                                      in_=embd8l[b])
            for i in range(L):
                twd = []
                twb = []
                for m in range(NCH):
                    t = twp.tile([P, T], BF16, tag="twp", name="twd")
                    nc.sync.dma_start(out=t, in_=TwD[i, m])
                    twd.append(t)
                for m in range(NCH):
                    t = twp.tile([P, T], BF16, tag="twp", name="twb")
                    nc.sync.dma_start(out=t, in_=TwB[i, m])
                    twb.append(t)
                al_t = w2p.tile([P, P], BF16, tag="w2p", name="al")
                nc.sync.dma_start(out=al_t, in_=AL4[i])
                air_t = w2p.tile([P, P], BF16, tag="w2p", name="air")
                nc.sync.dma_start(out=air_t, in_=AIR4[i])

                # ---- stage A ----
                for m in range(NCH):
                    wc = wp.tile([P, 22, 2, P], mybir.dt.float8e4, tag="wp",
                                 name="wc")
                    nc.sync.dma_start(out=wc, in_=Wconv8[i, m])
                    for b in range(BPC):
                        px = ps2.tile([P, T], F32, tag="ps", name="px")
                        pg = ps2.tile([P, T], F32, tag="ps", name="pg")
                        pr = ps2.tile([P, T], F32, tag="ps", name="pr")
                        for ch in range(NC_T):
                            t0 = ch * TC
                            for j, (tap, kcp) in enumerate(_PAIRS):
                                r8 = h8[b][:, kcp:kcp + 2,
                                           t0 + tap:t0 + tap + TC]
                                rl = h8l[b][:, kcp:kcp + 2,
                                            t0 + tap:t0 + tap + TC]
                                mm(px[:, t0:t0 + TC], wc[:, j], r8,
                                   j == 0, False, pm=PM)
                                mm(px[:, t0:t0 + TC], wc[:, j], rl,
                                   False, False, pm=PM)
                                mm(px[:, t0:t0 + TC], wc[:, 6 + j], r8,
                                   False, j == 5, pm=PM)
                            for j, (tap, kcp) in enumerate(_PAIRS):
                                r8 = h8[b][:, kcp:kcp + 2,
                                           t0 + tap:t0 + tap + TC]
                                mm(pg[:, t0:t0 + TC], wc[:, 12 + j], r8,
                                   j == 0, j == 5, pm=PM)
                            for j in range(2):
                                r8 = h8[b][:, 2 * j:2 * j + 2,
                                           t0 + 2:t0 + 2 + TC]
                                rl = h8l[b][:, 2 * j:2 * j + 2,
                                            t0 + 2:t0 + 2 + TC]
                                mm(pr[:, t0:t0 + TC], wc[:, 18 + j], r8,
                                   j == 0, False, pm=PM)
                                mm(pr[:, t0:t0 + TC], wc[:, 18 + j], rl,
                                   False, False, pm=PM)
                                mm(pr[:, t0:t0 + TC], wc[:, 20 + j], r8,
                                   False, False, pm=PM)
                            mm(pr[:, t0:t0 + TC], id_t,
                               twd[m][:, t0:t0 + TC], False, True)
                        sg = sgp.tile([P, T], BF16, tag="sgp", name="sg")
                        nc.scalar.activation(out=sg, in_=pg, func=AF.Sigmoid,
                                             bias=bid_col(i, m),
                                             scale=1.0 / CC)
                        t1 = t1p.tile([P, T], BF16, tag="t1p", name="t1")
                        nc.vector.scalar_tensor_tensor(
                            out=t1, in0=px, scalar=bglu_col(i, m), in1=sg,
                            op0=OP.add, op1=OP.mult)
                        nc.vector.tensor_tensor(cv[b][:, m, :], t1, pr,
                                                OP.add)

                # ---- stage B ----
                w2 = []
                for m in range(NCH):
                    t = w2p.tile([P, NCH, P], BF16, tag="w2p", name="w2")
                    nc.sync.dma_start(out=t, in_=W2e[i, m])
                    w2.append(t)
                for b in range(BPC):
                    for m in range(NCH):
                        pd = ps2.tile([P, T], F32, tag="ps", name="pd")
                        for ch in range(NC_T):
                            t0 = ch * TC
                            for kc in range(NCH):
                                mm(pd[:, t0:t0 + TC], w2[m][:, kc, :],
                                   cv[b][:, kc, t0:t0 + TC], kc == 0, False)
                            mm(pd[:, t0:t0 + TC], id_t, twb[m][:, t0:t0 + TC],
                               False, False)
                            mm(pd[:, t0:t0 + TC],
                               al_t[32 * m:32 * (m + 1), :],
                               oh_t[b][32 * m:32 * (m + 1), t0:t0 + TC],
                               False, True, tp=(32 * m, 0))
                        nc.scalar.copy(ddt[b][:, m, :], pd)
                    pden = ps2.tile([P, T], F32, tag="ps", name="pden")
                    for ch in range(NC_T):
                        t0 = ch * TC
                        for kc in range(NCH):
                            mm(pden[:, t0:t0 + TC], mrep[b][kc],
                               ddt[b][:, kc, t0:t0 + TC],
                               kc == 0, kc == NCH - 1)
                    nc.vector.reciprocal(out=rbt[b], in_=pden)
                    for kc in range(NCH):
                        eng = nc.vector if kc < 2 else nc.gpsimd
                        eng.tensor_tensor(dp[b][:, kc, :],
                                          ddt[b][:, kc, :], rbt[b],
                                          OP.mult)

                # ---- stage D ----
                gf = [[gfp.tile([P, 2, C], mybir.dt.float8e4, tag="gfp",
                                name="gf") for _ in range(2)]
                      for _ in range(BPC)]
                for b in range(BPC):
                    for pair in range(2):
                        nc.sync.dma_start(out=gf[b][pair],
                                          in_=gfd[i][b][pair])
                last = (i == L - 1)
                for m in range(NCH):
                    for b in range(BPC):
                        pc = ps2.tile([P, T], F32, tag="ps", name="pc")
                        for ch in range(NC_T):
                            t0 = ch * TC
                            for pair in range(2):
                                mm(pc[:, t0:t0 + TC],
                                   gf[b][pair][:, :, m * P:(m + 1) * P],
                                   dp[b][:, 2 * pair:2 * pair + 2,
                                         t0:t0 + TC],
                                   pair == 0, False,
                                   pm=mybir.MatmulPerfMode.DoubleRow)
                            mm(pc[:, t0:t0 + TC], idsw_t,
                               cv[b][:, m, t0:t0 + TC], False, False)
                            mm(pc[:, t0:t0 + TC],
                               air_t[32 * m:32 * (m + 1), :],
                               oh_t[b][32 * m:32 * (m + 1), t0:t0 + TC],
                               False, True, tp=(32 * m, 0))
                        nc.scalar.copy(h[b][:, m, 2:2 + T], pc)
                        if not last:
                            nc.vector.tensor_copy(
                                out=h8[b][:, m, 2:2 + T],
                                in_=h[b][:, m, 2:2 + T])
                            nc.vector.tensor_tensor(
                                h8l[b][:, m, 2:2 + T],
                                h[b][:, m, 2:2 + T],
                                h8[b][:, m, 2:2 + T], OP.subtract)

            # ---- output ----
            wo_t = w2p.tile([P, NCH, V], BF16, tag="w2p", name="wo")
            nc.sync.dma_start(out=wo_t, in_=Wout)
            wor_t = w2p.tile([P, NCH, V], BF16, tag="w2p", name="wor")
            nc.sync.dma_start(out=wor_t, in_=Woutres)
            for b in range(BPC):
                em = []
                for kd in range(NCH):
                    t = twp.tile([P, T], BF16, tag="twp", name="em")
                    nc.sync.dma_start(out=t, in_=embd[b][:, kd, :])
                    em.append(t)
                for ch in range(NC_T):
                    t0 = ch * TC
                    po = ps2.tile([V, TC], F32, tag="ps", name="po")
                    for kc in range(NCH):
                        mm(po, wo_t[:, kc, :], h[b][:, kc, 2 + t0:2 + t0 + TC],
                           kc == 0, False)
                    for kd in range(NCH):
                        mm(po, wor_t[:, kd, :], em[kd][:, t0:t0 + TC],
                           False, kd == NCH - 1)
                    ot = sgp.tile([V, TC], F32, tag="otp", name="ot")
                    nc.scalar.activation(out=ot, in_=po, func=AF.Identity,
                                         bias=bout_col, scale=1.0)
                    nc.sync.dma_start(out=out[b, :, t0:t0 + TC], in_=ot)

        es.close()

    nc.compile()
    return nc


def _block_lhsT(w):
    """(Cin, Cout) -> [kc, m, 128, 128] lhsT blocks."""
    ci, co = w.shape
    return np.ascontiguousarray(
        w.reshape(ci // P, P, co // P, P).transpose(0, 2, 1, 3))


def host_prep(inputs):
    import ml_dtypes
    BF = ml_dtypes.bfloat16
    F8 = ml_dtypes.float8_e4m3

    f = lambda x: np.asarray(x, dtype=np.float32)
    labels = np.asarray(inputs["labels"]).astype(np.int64)
    enc_seq = f(inputs["enc_seq"])
    lw = f(inputs["label_embed_W"])
    tw = f(inputs["time_embed_W"])
    wglu, bglu = f(inputs["conv_glu_w"]), f(inputs["conv_glu_b"])
    wid, bid = f(inputs["conv_id_w"]), f(inputs["conv_id_b"])
    wres, bres = f(inputs["res_proj_w"]), f(inputs["res_proj_b"])
    winres, binres = f(inputs["inres_w"]), f(inputs["inres_b"])
    w2e, b2e = f(inputs["in2enc_w"]), f(inputs["in2enc_b"])
    wl2e, bl2e = f(inputs["lab2enc_w"]), f(inputs["lab2enc_b"])
    we2i, be2i = f(inputs["enc2in_w"]), f(inputs["enc2in_b"])
    worr, borr = f(inputs["out_res_w"]), f(inputs["out_res_b"])
    wop, bop = f(inputs["out_proj_w"]), f(inputs["out_proj_b"])

    # conv pair blocks hi/lo: [L, NCH(m), 22, P, 2, P] fp8
    # order: x-Whi 0-5, x-Wlo 6-11, g-Whi 12-17, res-Whi 18-19, res-Wlo 20-21
    def pair_block(wsrc, i, m, tap, kcp):
        blk = np.zeros((P, 2, P), np.float32)
        for sl in range(2):
            kc = kcp + sl
            if wsrc.ndim == 4:
                blk[:, sl, :] = wsrc[i, m * P:(m + 1) * P,
                                     kc * P:(kc + 1) * P, tap].T * SW
            else:
                blk[:, sl, :] = wsrc[i, m * P:(m + 1) * P,
                                     kc * P:(kc + 1) * P].T * SW
        return blk

    def hilo(blk):
        hi = blk.astype(F8).astype(np.float32)
        lo = blk - hi
        return hi, lo

    Wconv8 = np.zeros((L, NCH, 22, P, 2, P), np.float32)
    for i in range(L):
        for m in range(NCH):
            xhi, xlo, ghi = [], [], []
            for tap, kcp in _PAIRS:
                hi, lo = hilo(pair_block(wglu, i, m, tap, kcp))
                xhi.append(hi); xlo.append(lo)
                ghi.append(pair_block(wid, i, m, tap, kcp))
            rhi, rlo = [], []
            for kcp in (0, 2):
                hi, lo = hilo(pair_block(wres, i, m, 0, kcp))
                rhi.append(hi); rlo.append(lo)
            Wconv8[i, m] = np.stack(xhi + xlo + ghi + rhi + rlo)
    if USE_SW_INTERLEAVE:
        # interleave+reverse along M for the first 12 (glu/id) and res too
        sw = np.empty_like(Wconv8)
        sw[..., 0::1, :, :] = Wconv8  # placeholder; overwritten below
        swv = Wconv8.reshape(L, NCH, 14, P, 2, P)
        inter = np.empty((L, NCH, 14, P, 2 * P), np.float32)
        inter[..., 0::2] = swv[..., 0, ::-1]
        inter[..., 1::2] = swv[..., 1, ::-1]
        Wconv8 = inter.reshape(L, NCH, 14, P, 2, P)
    Wconv8 = Wconv8.transpose(0, 1, 3, 2, 4, 5).copy().astype(F8)  # [L,m,P,14,2,P]

    twD_true = np.stack([tw[:T] @ winres[i].T + binres[i] + be2i[i]
                         for i in range(L)])                    # (L,T,C)
    twB_true = np.stack([tw[:T] @ wl2e[i].T + b2e[i] + bl2e[i]
                         - twD_true[i] @ w2e[i].T for i in range(L)])
    TwD = np.ascontiguousarray(
        (twD_true * CC).transpose(0, 2, 1).reshape(L, NCH, P, T)).astype(BF)
    TwB = np.ascontiguousarray(
        twB_true.transpose(0, 2, 1).reshape(L, NCH, P, T)).astype(BF)

    W2e_b = np.stack([
        _block_lhsT(w2e[i].T / CC).transpose(1, 2, 0, 3) for i in range(L)
    ]).astype(BF)                                               # [L,m,P,kc,P]

    AL4 = np.zeros((L, P, P), np.float32)
    AIR4 = np.zeros((L, P, P), np.float32)
    for i in range(L):
        ALi = lw @ wl2e[i].T                                    # (V,E)
        AIRi = lw @ winres[i].T * SH                            # (V,C)
        for m in range(NCH):
            AL4[i, 32 * m:32 * (m + 1), :] = ALi[:, m * P:(m + 1) * P]
            AIR4[i, 32 * m:32 * (m + 1), :] = AIRi[:, m * P:(m + 1) * P]
    AL4 = AL4.astype(BF)
    AIR4 = AIR4.astype(BF)

    Ident = np.eye(P, dtype=np.float32).astype(BF)
    IdentSW = (np.eye(P, dtype=np.float32) / SW).astype(BF)

    Wout = np.ascontiguousarray(
        (wop / SH).T.reshape(NCH, P, V).transpose(1, 0, 2)).astype(BF)
    Woutres = np.ascontiguousarray(
        worr.T.reshape(NCH, P, V).transpose(1, 0, 2)).astype(BF)
    timeTb = np.ascontiguousarray(tw[:T].T.reshape(NCH, P, T)).astype(BF)

    lab4 = np.zeros((P, P), np.float32)
    for kd in range(NCH):
        lab4[32 * kd:32 * (kd + 1), :] = lw[:, kd * P:(kd + 1) * P]
    lab4 = lab4.astype(BF)

    We2i_r = np.ascontiguousarray(
        np.stack([we2i[i].T * (SH / SDP) for i in range(L)])
        .reshape(L, NCH, P, C)).astype(BF)

    onesv = np.full((P, 2), 1.0 / SDP, np.float32).astype(BF)

    NBIAS = 2 * L * NCH + 1
    biasall = np.zeros((P, NBIAS), np.float32)
    for i in range(L):
        for m in range(NCH):
            biasall[:, i * NCH + m] = bid[i, m * P:(m + 1) * P]
            biasall[:, L * NCH + i * NCH + m] = \
                bglu[i, m * P:(m + 1) * P] * CC
    biasall[:V, NBIAS - 1] = bop + borr

    shared = dict(Wconv8=Wconv8, W2e=W2e_b, TwB=TwB, TwD=TwD, AL4=AL4,
                  AIR4=AIR4, Ident=Ident, IdentSW=IdentSW, Wout=Wout,
                  Woutres=Woutres, timeTb=timeTb, lab4=lab4,
                  We2i_r=We2i_r, onesv=onesv, biasall=biasall)

    in_maps = []
    for cidx in range(NCORES):
        bsel = [cidx * BPC + p for p in range(BPC)]
        oh = np.zeros((BPC, P, T), np.float32)
        for p, bb in enumerate(bsel):
            onehot = np.zeros((V, T), np.float32)
            onehot[labels[:, bb], np.arange(T)] = 1.0
            for s in range(NCH):
                oh[p, 32 * s:32 * (s + 1), :] = onehot
        enc_r = np.stack([
            np.ascontiguousarray(enc_seq[:, bb, :]).reshape(NCH, P, E)
            for bb in bsel])
        m = dict(shared)
        m.update(oh4=oh.astype(BF), enc_r=enc_r.astype(BF))
        in_maps.append(m)
    return in_maps


def get_compiled():
    global _compiled
    if _compiled is None:
        _compiled = _build_nc()
    return _compiled


def kernel(**inputs):
    from concourse.bass_utils import run_bass_kernel_spmd

    nc = get_compiled()
    in_maps = host_prep(inputs)
    res = run_bass_kernel_spmd(nc, in_maps, list(range(NCORES)))
    out = np.empty((T, B, V), np.float32)
    for c in range(NCORES):
        o = res.results[c]["out"]
        for p in range(BPC):
            out[:, c * BPC + p, :] = o[p].T
    return out
